# revision 18
# baseline (speedup 1.0000x reference)
"""Trainium2 Bass kernel for nn_AutoEncoder (segment_reduce).

6-layer MLP autoencoder on a single 16384-vector + segmented softmax over
1024 contiguous segments, distributed over 8 NeuronCores.

Sharding (per core c of 8):
  L1: W1 column-shard (16384 x 256)  -> h1 shard (256), no collective
  L2: W2 row-shard    (256 x 512)    -> partial h2, AllReduce #1 (2KB)
  L3: W3 replicated   (512 x 128)    -> z (128)
  L4: W4 replicated   (128 x 512)    -> h4 (512)
  L5: W5 column-shard (512 x 256)    -> h5 shard (256), no collective
  L6: W6 ROW-shard    (256 x 16384)  -> partial y,  AllReduce #2 (64KB)
  softmax: replicated on every core (full y), two segmented prefix scans
  (forward + reversed halo layout) on the vector engine; b6/8 is folded
  into each core's partial-y drain so AllReduce #2 sums it exactly once.

All bulk + latency-critical DMAs ride the single SWDGE (gpsimd) queue, which
drains strictly in emission order; HWDGE transfers starve behind SWDGE bulk
on this machine, so only early constants use the sync ring. W1/W6 are cast
f32->bf16 inline by the DMA (HBM still reads the full f32 bytes; TensorE runs
4x faster on bf16). A dummy warm-up collective absorbs ncfw's ~11us first-
collective wake latency in the shadow of the W1 stream.
"""

import sys

if "/opt/trn_rl_repo" not in sys.path:
    sys.path.insert(0, "/opt/trn_rl_repo")

import numpy as np

import concourse.bass as bass
import concourse.mybir as mybir
import concourse.tile as tile
from concourse.bass_utils import run_bass_kernel_spmd
from concourse.tile_rust import add_dep_helper

DS = 16384
H1, H2, H3 = 2048, 512, 128
NC = 8
C1 = H1 // NC   # 256  h1 / h5 shard
C6 = DS // NC   # 2048
F32 = mybir.dt.float32
BF16 = mybir.dt.bfloat16


def _split_sync_waits(nc):
    """The walrus build in this env only allows 1 sync wait on CTRL-class
    instructions (Drain/NoOp). Tile's tail drain carries one wait per live
    semaphore lane. Split excess waits onto preceding single-wait NOPs."""
    for f in nc.m.functions:
        for b in f.blocks:
            new_insts = []
            for inst in b.instructions:
                si = inst.sync_info
                if si is not None and si.on_wait and len(si.on_wait) > 1:
                    waits = list(si.on_wait)
                    head, tail = waits[:-1], waits[-1:]
                    for i, w in enumerate(head):
                        new_insts.append(
                            mybir.InstNoOp(
                                name=f"{inst.name}-ws{i}",
                                engine=inst.engine,
                                bass_nofuse=True,
                                sync_info=mybir.SyncInfo(on_wait=[w], on_update=[]),
                            )
                        )
                    si.on_wait = tail
                new_insts.append(inst)
            b.instructions = new_insts


def build_graph():
    nc = bass.Bass()
    P = nc.declare_dram_parameter
    x_in = P("x", [DS], F32, isOutput=False)
    w1 = P("w1", [2, 128, 64 * 256], F32, isOutput=False)   # (chunk, p, tl*256+n)
    w2 = P("w2", [128, 2 * 512], F32, isOutput=False)
    w3 = P("w3", [128, 4 * 128], F32, isOutput=False)
    w4 = P("w4", [128, 512], F32, isOutput=False)
    w5 = P("w5", [128, 4 * 256], F32, isOutput=False)
    w6 = P("w6", [4, 128, 4 * 2048], F32, isOutput=False)    # (q, p, jj*4096+k*2048+n)
    b1cp = P("b1c", [128, 2], F32, isOutput=False)
    b2c = P("b2c", [128, 4], F32, isOutput=False)
    b3c = P("b3c", [128, 1], F32, isOutput=False)
    b4c = P("b4c", [128, 4], F32, isOutput=False)
    b5c = P("b5c", [128, 2], F32, isOutput=False)
    b6s8 = P("b6s8", [8, 2048], F32, isOutput=False)        # b6/8 rows
    eye = P("eye", [128, 128], F32, isOutput=False)
    jrev = P("jrev", [128, 128], F32, isOutput=False)
    shm = P("shm", [128, 128], F32, isOutput=False)
    mf_in = P("mf", [128, 256], F32, isOutput=False)
    mr_in = P("mr", [128, 256], F32, isOutput=False)
    out_ext = P("out", [DS], F32, isOutput=True)

    Tanh = mybir.ActivationFunctionType.Tanh
    Iden = mybir.ActivationFunctionType.Identity
    Exp = mybir.ActivationFunctionType.Exp
    ADD = mybir.AluOpType.add
    SUB = mybir.AluOpType.subtract
    MUL = mybir.AluOpType.mult
    BYP = mybir.AluOpType.bypass
    RG = [list(range(NC))]

    with tile.TileContext(nc) as tc:
        with (
            tc.tile_pool(name="const", bufs=1) as cp,
            tc.tile_pool(name="w1p", bufs=2) as w1p,
            tc.tile_pool(name="w6p", bufs=4) as w6p,
            tc.tile_pool(name="act", bufs=1) as ap,
            tc.tile_pool(name="psA", bufs=4, space="PSUM") as psA,
            tc.tile_pool(name="dram", bufs=1, space="DRAM") as dp,
        ):
            # ---- small constants via the sync (HWDGE) ring, front-loaded ----
            eyesb = cp.tile([128, 128], F32)
            nc.sync.dma_start(eyesb[:], eye[:])
            jsb = cp.tile([128, 128], F32)
            nc.sync.dma_start(jsb[:], jrev[:])
            shsb = cp.tile([128, 128], F32)
            nc.sync.dma_start(shsb[:], shm[:])
            mf = cp.tile([128, 256], F32)
            nc.sync.dma_start(mf[:], mf_in[:])
            mr = cp.tile([128, 256], F32)
            nc.sync.dma_start(mr[:], mr_in[:])
            b6sb8 = cp.tile([8, 2048], F32)
            nc.sync.dma_start(b6sb8[:], b6s8[:])
            b1c = cp.tile([128, 2], F32)
            nc.sync.dma_start(b1c[:], b1cp[:])
            b2sb = cp.tile([128, 4], F32)
            nc.sync.dma_start(b2sb[:], b2c[:])
            b3sb = cp.tile([128, 1], F32)
            nc.sync.dma_start(b3sb[:], b3c[:])
            b4sb = cp.tile([128, 4], F32)
            nc.sync.dma_start(b4sb[:], b4c[:])
            b5sb = cp.tile([128, 2], F32)
            nc.sync.dma_start(b5sb[:], b5c[:])
            # middle weights (sync ring; small, needed ~55us in)
            w2sb = cp.tile([128, 1024], F32)
            nc.sync.dma_start(w2sb[:], w2[:])
            w3sb = cp.tile([128, 512], F32)
            nc.sync.dma_start(w3sb[:], w3[:])
            w4sb = cp.tile([128, 512], F32)
            nc.sync.dma_start(w4sb[:], w4[:])
            w5sb = cp.tile([128, 1024], F32)
            nc.sync.dma_start(w5sb[:], w5[:])
            # ---- SWDGE queue head: x, then the 8 W1 cast-chunks ----
            x2d = cp.tile([128, 128], F32)
            nc.gpsimd.dma_start(x2d[:], x_in[:].rearrange("(a b) -> a b", b=128))
            w1sb_l = []
            for c in range(2):
                w1sb = w1p.tile([128, 64 * 256], BF16, tag="w1sb", name="w1sb")
                nc.gpsimd.dma_start(w1sb[:], w1[c])
                w1sb_l.append(w1sb)

            # warm-up collective: absorbs ncfw's first-collective wake latency
            # inside the W1 stream window (blocks Pool ~12us, nothing waits)
            dumin = dp.tile([8], F32)
            dumout = dp.tile([8 * NC], F32, addr_space="Shared")
            nc.gpsimd.collective_compute(
                "AllGather", BYP, ins=[dumin[:].opt()], outs=[dumout[:].opt()],
                replica_groups=RG,
            )

            # ---- x -> xT columns (bf16) ----
            ps_xt = psA.tile([128, 128], F32, tag="psA")
            nc.tensor.matmul(ps_xt[:], x2d[:], eyesb[:], start=True, stop=True)
            xT = cp.tile([128, 128], BF16)
            nc.vector.tensor_copy(xT[:], ps_xt[:])

            # ---- L1 (weight-stationary): h1 shard as psum columns ----
            ps1c = [
                psA.tile([128, 1], F32, tag="psA", name=f"ps1c{m}") for m in range(2)
            ]
            for c in range(2):
                for tl in range(64):
                    t = 64 * c + tl
                    for m in range(2):
                        nc.tensor.matmul(
                            ps1c[m][:],
                            w1sb_l[c][:, tl * 256 + 128 * m : tl * 256 + 128 * (m + 1)],
                            xT[:, t : t + 1],
                            start=(t == 0),
                            stop=(t == 127),
                        )
            h1cols = ap.tile([128, 2], F32)
            for m in range(2):
                nc.scalar.activation(
                    h1cols[:, m : m + 1], ps1c[m][:], Tanh, bias=b1c[:, m : m + 1]
                )

            # ---- L2 partials -> AllReduce #1 ----
            p2sb = ap.tile([128, 4], F32)
            for m in range(4):
                pm = psA.tile([128, 1], F32, tag="psA", name="pm2")
                for k in range(2):
                    nc.tensor.matmul(
                        pm[:],
                        w2sb[:, k * 512 + 128 * m : k * 512 + 128 * (m + 1)],
                        h1cols[:, k : k + 1],
                        start=(k == 0),
                        stop=(k == 1),
                    )
                nc.vector.tensor_copy(p2sb[:, m : m + 1], pm[:])
            arin = dp.tile([H2], F32)
            nc.gpsimd.dma_start(arin[:].rearrange("(a b) -> a b", a=128), p2sb[:])
            arout = dp.tile([H2], F32, addr_space="Shared")
            nc.gpsimd.collective_compute(
                "AllReduce", ADD, ins=[arin[:].opt()], outs=[arout[:].opt()],
                replica_groups=RG,
            )
            h2pre = ap.tile([128, 4], F32)
            d_h2pre = nc.gpsimd.dma_start(
                h2pre[:], arout[:].rearrange("(a b) -> a b", a=128)
            )
            # W6 streams only after the AR readback is issued: collective
            # traffic starves behind SWDGE bulk, so the queue must be clear
            # for the whole AllReduce. sync=False keeps scheduler order
            # without gating on the readback's data.
            w6sb = []
            for t in range(4):
                t6 = w6p.tile([128, 4 * 2048], BF16, tag="w6", name=f"w6sb{t}")
                d = nc.gpsimd.dma_start(t6[:], w6[t])
                add_dep_helper(d.ins, d_h2pre.ins, sync=False,
                               reason="W6 stream after AR readback issue")
                w6sb.append(t6)
            h2cols = ap.tile([128, 4], F32)
            for m in range(4):
                nc.scalar.activation(
                    h2cols[:, m : m + 1], h2pre[:, m : m + 1], Tanh,
                    bias=b2sb[:, m : m + 1],
                )

            # ---- L3: z = h2 @ W3 + b3 (no tanh) ----
            pz = psA.tile([128, 1], F32, tag="psA", name="pz")
            for k in range(4):
                nc.tensor.matmul(
                    pz[:], w3sb[:, 128 * k : 128 * (k + 1)], h2cols[:, k : k + 1],
                    start=(k == 0), stop=(k == 3),
                )
            zcol = ap.tile([128, 1], F32)
            nc.scalar.activation(zcol[:], pz[:], Iden, bias=b3sb[:])

            # ---- L4: h4 = tanh(z @ W4 + b4) ----
            h4cols = ap.tile([128, 4], F32)
            for m in range(4):
                pm = psA.tile([128, 1], F32, tag="psA", name="pm4")
                nc.tensor.matmul(
                    pm[:], w4sb[:, 128 * m : 128 * (m + 1)], zcol[:],
                    start=True, stop=True,
                )
                nc.scalar.activation(
                    h4cols[:, m : m + 1], pm[:], Tanh, bias=b4sb[:, m : m + 1]
                )

            # ---- L5: h5 shard (bf16 columns, ready for L6) ----
            h5colsb = ap.tile([128, 2], BF16)
            for m in range(2):
                pm = psA.tile([128, 1], F32, tag="psA", name="pm5")
                for k in range(4):
                    nc.tensor.matmul(
                        pm[:],
                        w5sb[:, k * 256 + 128 * m : k * 256 + 128 * (m + 1)],
                        h4cols[:, k : k + 1],
                        start=(k == 0),
                        stop=(k == 3),
                    )
                nc.scalar.activation(
                    h5colsb[:, m : m + 1], pm[:], Tanh, bias=b5sb[:, m : m + 1]
                )

            # ---- L6 row-shard: j-group outputs on distinct PSUM partitions
            # via a sliding zero-padded lhsT window; one [8,2048] accumulator ----
            bufk = ap.tile([128, 16 + 14], BF16)
            nc.vector.memset(bufk[:], 0.0)
            nc.vector.tensor_copy(bufk[:, 7 : 8], h5colsb[:, 0:1])
            nc.vector.tensor_copy(bufk[:, 22 : 23], h5colsb[:, 1:2])
            ps6 = psA.tile([8, 2048], F32, tag="ps6big", bufs=1, name="ps6big")
            for j in range(8):
                q, jj = j // 2, j % 2
                for k in range(2):
                    lhs = bufk[:, 15 * k + 7 - j : 15 * k + 15 - j]
                    for nb in range(4):
                        off = 4096 * jj + 2048 * k + 512 * nb
                        nc.tensor.matmul(
                            ps6[:, 512 * nb : 512 * (nb + 1)],
                            lhs,
                            w6sb[q][:, off : off + 512],
                            start=(j == 0 and k == 0),
                            stop=(j == 7 and k == 1),
                        )
            ys8 = ap.tile([8, 2048], F32)
            nc.vector.tensor_tensor(ys8[:], ps6[:], b6sb8[:], ADD)
            ag2in = dp.tile([DS], F32)
            nc.gpsimd.dma_start(
                ag2in[:].rearrange("(a b) -> a b", b=2048), ys8[:]
            )

            # ---- AllReduce #2: y = sum of partials (64KB) ----
            yfull = dp.tile([DS], F32, addr_space="Shared")
            nc.gpsimd.collective_compute(
                "AllReduce", ADD, ins=[ag2in[:].opt()], outs=[yfull[:].opt()],
                replica_groups=RG,
            )

            # ---- segmented softmax on full y (replicated), b6 via halo ----
            hf = ap.tile([128, 256], F32)
            nc.gpsimd.dma_start(
                hf[:, 128:256], yfull[:].rearrange("(a b) -> a b", b=128)
            )
            nc.gpsimd.dma_start(
                hf[1:128, 0:128],
                yfull[0 : 127 * 128].rearrange("(a b) -> a b", b=128),
            )
            nc.vector.memset(hf[0:1, 0:128], -1e30)
            hfe = ap.tile([128, 256], F32)
            nc.scalar.activation(hfe[:], hf[:], Exp)
            sf = ap.tile([128, 256], F32)
            nc.vector.tensor_tensor_scan(sf[:], mf[:], hfe[:], 0.0, MUL, ADD)

            e_ap = hfe[:, 128:256]
            pt1 = psA.tile([128, 128], F32, tag="psA", name="pt1")
            nc.tensor.transpose(pt1[:], e_ap, jsb[:])
            ct1 = ap.tile([128, 128], F32)
            nc.vector.tensor_copy(ct1[:], pt1[:])
            pt2 = psA.tile([128, 128], F32, tag="psA", name="pt2")
            nc.tensor.transpose(pt2[:], ct1[:], jsb[:])
            er = ap.tile([128, 128], F32)
            nc.vector.tensor_copy(er[:], pt2[:])
            psh = psA.tile([128, 128], F32, tag="psA", name="psh")
            nc.tensor.matmul(psh[:], shsb[:], er[:], start=True, stop=True)
            sr1 = ap.tile([128, 128], F32)
            nc.vector.tensor_tensor_scan(sr1[:], mr[:, 0:128], psh[:], 0.0, MUL, ADD)
            sr = ap.tile([128, 128], F32)
            nc.vector.tensor_tensor_scan(
                sr[:], mr[:, 128:256], er[:], sr1[:, 127:128], MUL, ADD
            )
            pt3 = psA.tile([128, 128], F32, tag="psA", name="pt3")
            nc.tensor.transpose(pt3[:], sr[:], jsb[:])
            ct3 = ap.tile([128, 128], F32)
            nc.vector.tensor_copy(ct3[:], pt3[:])
            pt4 = psA.tile([128, 128], F32, tag="psA", name="pt4")
            nc.tensor.transpose(pt4[:], ct3[:], jsb[:])
            dd = ap.tile([128, 128], F32)
            nc.vector.tensor_tensor(dd[:], sf[:, 128:256], pt4[:], ADD)
            nc.vector.tensor_tensor(dd[:], dd[:], e_ap, SUB)
            rr = ap.tile([128, 128], F32)
            nc.vector.reciprocal(rr[:], dd[:])
            outt = ap.tile([128, 128], F32)
            nc.vector.tensor_tensor(outt[:], e_ap, rr[:], MUL)
            nc.gpsimd.dma_start(
                out_ext[:].rearrange("(a b) -> a b", b=128), outt[:]
            )

    _split_sync_waits(nc)
    return nc


def _prep_inputs(x, W1, b1, W2, b2, W3, b3, W4, b4, W5, b5, W6, b6, segment_ids):
    """Host-side sharding + layout permutation. Returns in_maps (one per core)."""
    x = np.ascontiguousarray(x, np.float32)
    seg = np.asarray(segment_ids)

    start = np.ones(DS, bool)
    start[1:] = seg[1:] != seg[:-1]
    end = np.ones(DS, bool)
    end[:-1] = seg[:-1] != seg[1:]
    seg_len = np.diff(np.concatenate([np.where(start)[0], [DS]]))
    assert seg_len.max() <= 128, f"segment too long for halo scan: {seg_len.max()}"
    m_own_f = (~start).astype(np.float32)
    mf = np.zeros((128, 256), np.float32)
    mf[1:, 0:128] = m_own_f.reshape(128, 128)[0:127, :]
    mf[:, 128:256] = m_own_f.reshape(128, 128)
    m_own_r = (~end).astype(np.float32)[::-1].copy()
    mr = np.zeros((128, 256), np.float32)
    mr[1:, 0:128] = m_own_r.reshape(128, 128)[0:127, :]
    mr[:, 128:256] = m_own_r.reshape(128, 128)

    b6s8 = (np.asarray(b6, np.float32) / 8.0).reshape(8, 2048)

    eye = np.eye(128, dtype=np.float32)
    jrev = eye[::-1].copy()
    shm = np.zeros((128, 128), np.float32)
    shm[np.arange(127), np.arange(1, 128)] = 1.0

    b2cv = np.ascontiguousarray(np.asarray(b2, np.float32).reshape(4, 128).T)
    b3cv = np.ascontiguousarray(np.asarray(b3, np.float32).reshape(1, 128).T)
    b4cv = np.ascontiguousarray(np.asarray(b4, np.float32).reshape(4, 128).T)

    W1 = np.asarray(W1, np.float32)
    W2 = np.asarray(W2, np.float32)
    W3 = np.asarray(W3, np.float32)
    W4 = np.asarray(W4, np.float32)
    W5 = np.asarray(W5, np.float32)
    W6 = np.asarray(W6, np.float32)

    w3h = np.ascontiguousarray(
        W3.reshape(4, 128, H3).transpose(1, 0, 2).reshape(128, 4 * H3)
    )
    w4h = np.ascontiguousarray(W4)

    in_maps = []
    for c in range(NC):
        w1s = W1[:, C1 * c : C1 * (c + 1)]
        w1h = np.ascontiguousarray(
            w1s.reshape(2, 64, 128, C1).transpose(0, 2, 1, 3).reshape(2, 128, 64 * C1)
        )
        w2s = W2[C1 * c : C1 * (c + 1), :]
        w2h = np.ascontiguousarray(
            w2s.reshape(2, 128, H2).transpose(1, 0, 2).reshape(128, 2 * H2)
        )
        w5s = W5[:, C1 * c : C1 * (c + 1)]
        w5h = np.ascontiguousarray(
            w5s.reshape(4, 128, C1).transpose(1, 0, 2).reshape(128, 4 * C1)
        )
        # L6 row shard: rows [256c, 256c+256) of W6, chunked (j, k)
        w6s = W6[C1 * c : C1 * (c + 1), :]
        w6jk = w6s.reshape(2, 128, 8, 2048).transpose(2, 0, 1, 3)   # (j, k, 128, n)
        w6h = np.ascontiguousarray(
            w6jk.reshape(4, 2, 2, 128, 2048).transpose(0, 3, 1, 2, 4).reshape(4, 128, 4 * 2048)
        )
        b5s = np.asarray(b5, np.float32)[C1 * c : C1 * (c + 1)]
        in_maps.append(
            {
                "x": x,
                "w1": w1h,
                "w2": w2h,
                "w3": w3h,
                "w4": w4h,
                "w5": w5h,
                "w6": w6h,
                "b1c": np.ascontiguousarray(np.asarray(b1, np.float32)[C1 * c : C1 * (c + 1)].reshape(2, 128).T),
                "b2c": b2cv,
                "b3c": b3cv,
                "b4c": b4cv,
                "b5c": np.ascontiguousarray(b5s.reshape(2, 128).T),
                "b6s8": b6s8,
                "eye": eye,
                "jrev": jrev,
                "shm": shm,
                "mf": mf,
                "mr": mr,
            }
        )
    return in_maps


_GRAPH_CACHE = {}


def _get_graph():
    if "nc" not in _GRAPH_CACHE:
        _GRAPH_CACHE["nc"] = build_graph()
    return _GRAPH_CACHE["nc"]


def kernel(**inputs) -> np.ndarray:
    in_maps = _prep_inputs(**inputs)
    nc = _get_graph()
    res = run_bass_kernel_spmd(nc, in_maps, core_ids=list(range(NC)))
    return np.asarray(res.results[0]["out"], np.float32)
